# revision 7
# baseline (speedup 1.0000x reference)
"""Trainium2 Bass kernel for nn_Attention_15109694948028.

Single-layer attention block: QKV proj -> 8-head SDPA (S=4096, d_k=64)
-> out proj -> residual -> LayerNorm.  fp32 I/O.

Sharding: sequence-parallel across 8 NeuronCores with a K/V AllGather.
Core i owns query rows [i*512, (i+1)*512).  It computes K^T and V only
for its OWN 512 k-rows (1/8 of the work), pushes them to an internal
DRAM bounce buffer, and an 8-core AllGather (runs on TOPSP/SDMA --
free wrt the compute engines) distributes the full K^T/V.  Attention
is then pair-major: for each head-pair, all 32 k-tiles accumulate
ctx directly in PSUM (no SBUF ctx accumulator / no per-round adds).

exp is split between ScalarE (exact, activation Exp) and VectorE via a
one-instruction Schraudolph trick: Q is pre-scaled on the host by
184.664 = 0.125 * 1024/ln2 (0.125 = 1/sqrt(d_k)), so scores arrive in
PSUM already in "fp16-bit units"; DVE computes
   es_bits_u16 = max(scores + (15360 - 44.5), 0)
which, viewed as fp16, is exp(scores/184.664) to ~3% max / 1.8% mean
element error (softmax-averaging shrinks this ~10-30x in ctx).
The ScalarE half uses activation(Exp, scale=ln2/1024) and is exact.

All matmul operands are fp16 (same PE rate as bf16, 3 extra mantissa
bits); out-proj in float32r; residual/LayerNorm in fp32.

Benchmarking support: _build_nc(bench_reps=N) emits the whole body N
times unrolled (collectives cannot live inside For_i); test.py times
(T(reps_hi) - T(reps_lo)) / (reps_hi - reps_lo).
"""

import numpy as np

import concourse.bacc as bacc
import concourse.tile as tile
from concourse import mybir
from concourse.bass_utils import run_bass_kernel_spmd

f32 = mybir.dt.float32
f32r = mybir.dt.float32r
f16 = mybir.dt.float16
u16 = mybir.dt.uint16
AF = mybir.ActivationFunctionType
ALU = mybir.AluOpType

S = 4096
D = 512
H = 8
DK = 64
NCORES = 8
SLICE = S // NCORES          # 512 query rows per core
P = 128
NKT = S // P                 # 32 k-tiles
NQS = SLICE // P             # 4 q-subtiles
EPS = 1e-5

# Q pre-scale: folds 1/sqrt(d_k) and the fp16 exp-trick slope into Wq.
SCPRE = 0.125 * 1024.0 / np.log(2.0)        # 184.6635
ACT_SCALE = float(np.log(2.0) / 1024.0)     # exp(scores * this) on ScalarE
EXP_B = 15360.0 - 44.5                      # fp16 Schraudolph offset

# which k-tiles use ScalarE for exp (rest use the DVE trick)
_N_ACT = 18
_EXPA = [(t * _N_ACT) // 32 != ((t + 1) * _N_ACT) // 32 for t in range(32)]

# benchmarking ablation: skip the AllGather (ktq/vpq read the garbage
# kv_all directly) so the body is control-flow-free and can sit inside
# tc.For_i for precise differential timing.  The graded path always
# runs with the collective.
_NO_AG = False


def _round_f32r(a: np.ndarray) -> np.ndarray:
    b = np.ascontiguousarray(a, dtype=np.float32).view(np.uint32)
    lsb = (b >> np.uint32(12)) & np.uint32(1)
    return ((b + np.uint32(0x7FF) + lsb) & np.uint32(0xFFFFF000)).view(np.float32)


def _build_nc(has_bias: bool, has_bo: bool, has_gamma: bool, has_beta: bool,
              bench_reps: int = 0):
    nc = bacc.Bacc("TRN2", target_bir_lowering=False, debug=False)

    xq = nc.dram_tensor("xq", [D + 1, SLICE], f16, kind="ExternalInput")
    wq = nc.dram_tensor("wq", [D + 1, D], f16, kind="ExternalInput")
    wk = nc.dram_tensor("wk", [D + 1, D], f16, kind="ExternalInput")
    wv = nc.dram_tensor("wv", [D + 1, D], f16, kind="ExternalInput")
    wo = nc.dram_tensor("wo", [D, D], f32r, kind="ExternalInput")
    xs = nc.dram_tensor("x_slice", [SLICE, D], f32, kind="ExternalInput")
    bo = nc.dram_tensor("bo", [1, D], f32, kind="ExternalInput")
    gamma = nc.dram_tensor("gamma", [1, D], f32, kind="ExternalInput")
    beta = nc.dram_tensor("beta", [1, D], f32, kind="ExternalInput")
    y = nc.dram_tensor("y", [SLICE, D], f32, kind="ExternalOutput")

    # AllGather landing buffer: rank r rows [r*1024, r*1024+512) = K^T
    # slice (dims x k_local), rows [r*1024+512, (r+1)*1024) = V slice
    # (k_local x dims).  One Shared tensor per rep (collectives want
    # distinct known buffers).
    kv_alls = [
        nc.dram_tensor(f"kv_all{i}", [NCORES * 2 * D, SLICE], f16,
                       kind="Internal", addr_space="Shared")
        for i in range(max(1, bench_reps))
    ]

    with tile.TileContext(nc) as tc:
        def emit_body(kv_all):
            with (
                tc.tile_pool(name="dram", bufs=1, space="DRAM") as drp,
                tc.tile_pool(name="consts", bufs=1) as cp,
                tc.tile_pool(name="xqp", bufs=1) as xqp,
                tc.tile_pool(name="qt", bufs=1) as qtp,
                tc.tile_pool(name="ktq", bufs=1) as ktqp,
                tc.tile_pool(name="vpq", bufs=1) as vpqp,
                tc.tile_pool(name="es", bufs=3) as esp,
                tc.tile_pool(name="nrm", bufs=2) as nrmp,
                tc.tile_pool(name="ctxT", bufs=1) as ctp,
                tc.tile_pool(name="wts", bufs=1) as wtp,
            ):
                eps_t = cp.tile([P, 1], f32, tag="eps")
                nc.gpsimd.memset(eps_t[:], EPS)

                def bcast_row(dram_row, tag):
                    r = cp.tile([1, D], f32, tag=f"{tag}_row", name=f"{tag}_row")
                    nc.sync.dma_start(r[:], dram_row)
                    b = cp.tile([P, D], f32, tag=f"{tag}_b", name=f"{tag}_b")
                    nc.gpsimd.partition_broadcast(b[:], r[0:1, :])
                    return b

                bo_b = bcast_row(bo[:], "bo") if has_bo else None
                gamma_b = bcast_row(gamma[:], "gamma") if has_gamma else None
                beta_b = bcast_row(beta[:], "beta") if has_beta else None

                xqS = [xqp.tile([P, SLICE], f16, tag=f"xq{e}", name=f"xq{e}")
                       for e in range(4)]
                for e in range(4):
                    nc.sync.dma_start(xqS[e][:], xq[e * P:(e + 1) * P, :])
                if has_bias:
                    xq_ones = xqp.tile([1, SLICE], f16, tag="xq_ones")
                    nc.sync.dma_start(xq_ones[:], xq[D:D + 1, :])

                wkS = [wtp.tile([P, D], f16, tag=f"wk{e}", name=f"wk{e}")
                       for e in range(4)]
                wvS = [wtp.tile([P, D], f16, tag=f"wv{e}", name=f"wv{e}")
                       for e in range(4)]
                wqS = [wtp.tile([P, D], f16, tag=f"wq{e}", name=f"wq{e}")
                       for e in range(4)]
                for e in range(4):
                    nc.sync.dma_start(wkS[e][:], wk[e * P:(e + 1) * P, :])
                    nc.sync.dma_start(wvS[e][:], wv[e * P:(e + 1) * P, :])
                    nc.sync.dma_start(wqS[e][:], wq[e * P:(e + 1) * P, :])
                if has_bias:
                    wkb = wtp.tile([1, D], f16, tag="wkb")
                    wvb = wtp.tile([1, D], f16, tag="wvb")
                    wqb = wtp.tile([1, D], f16, tag="wqb")
                    nc.sync.dma_start(wkb[:], wk[D:D + 1, :])
                    nc.sync.dma_start(wvb[:], wv[D:D + 1, :])
                    nc.sync.dma_start(wqb[:], wq[D:D + 1, :])
                woS = [wtp.tile([DK, D], f32r, tag=f"wo{h}", name=f"wo{h}")
                       for h in range(H)]
                for h in range(H):
                    nc.sync.dma_start(woS[h][:], wo[h * DK:(h + 1) * DK, :])

                qtS = [qtp.tile([P, SLICE], f16, tag=f"qt{p}", name=f"qt{p}")
                       for p in range(4)]
                ktq = [ktqp.tile([P, S], f16, tag=f"ktq{p}", name=f"ktq{p}")
                       for p in range(4)]
                vpq = [vpqp.tile([P, H * 65], f16, tag=f"vpq{t}",
                                 name=f"vpq{t}") for t in range(NKT)]
                ctxT = [ctp.tile([DK, SLICE], f32r, tag=f"ctxT{h}",
                                 name=f"ctxT{h}") for h in range(H)]

                kv_loc = drp.tile([2 * D, SLICE], f16, tag="kv_loc")

                # ---- phase B0: local K^T and V slices -> DRAM -> AllGather
                with (
                    tc.tile_pool(name="psB", bufs=2, space="PSUM") as psB,
                    tc.tile_pool(name="stage", bufs=1) as stg,
                ):
                    for p in range(4):
                        ps = psB.tile([P, SLICE], f32, tag="psB")
                        for e in range(4):
                            nc.tensor.matmul(
                                ps[:], wkS[e][:, p * P:(p + 1) * P], xqS[e][:],
                                start=(e == 0), stop=(e == 3 and not has_bias))
                        if has_bias:
                            nc.tensor.matmul(ps[:], wkb[0:1, p * P:(p + 1) * P],
                                             xq_ones[:], start=False, stop=True)
                        kt_s = stg.tile([P, SLICE], f16, tag=f"kts{p}",
                                        name=f"kts{p}")
                        nc.vector.tensor_copy(kt_s[:], ps[:])
                        nc.sync.dma_start(kv_loc[p * P:(p + 1) * P, :], kt_s[:])
                    for t in range(4):
                        ps = psB.tile([P, D], f32, tag="psB")
                        for e in range(4):
                            nc.tensor.matmul(
                                ps[:], xqS[e][:, t * P:(t + 1) * P], wvS[e][:],
                                start=(e == 0), stop=(e == 3 and not has_bias))
                        if has_bias:
                            nc.tensor.matmul(
                                ps[:], xq_ones[0:1, t * P:(t + 1) * P], wvb[:],
                                start=False, stop=True)
                        v_s = stg.tile([P, D], f16, tag=f"vts{t}",
                                       name=f"vts{t}")
                        nc.vector.tensor_copy(v_s[:], ps[:])
                        nc.sync.dma_start(kv_loc[D + t * P:D + (t + 1) * P, :],
                                          v_s[:])

                    if not _NO_AG:
                        nc.gpsimd.collective_compute(
                            "AllGather", ALU.bypass,
                            replica_groups=[list(range(NCORES))],
                            ins=[kv_loc[:]], outs=[kv_all[:]],
                        )

                    # ---- phase A: Q^T projection (overlaps the AllGather)
                    for p in range(4):
                        ps = psB.tile([P, SLICE], f32, tag="psB")
                        for e in range(4):
                            nc.tensor.matmul(
                                ps[:], wqS[e][:, p * P:(p + 1) * P], xqS[e][:],
                                start=(e == 0), stop=(e == 3 and not has_bias))
                        if has_bias:
                            nc.tensor.matmul(ps[:], wqb[0:1, p * P:(p + 1) * P],
                                             xq_ones[:], start=False, stop=True)
                        nc.vector.tensor_copy(qtS[p][:], ps[:])

                # ---- gathered K^T / V -> SBUF working layout
                kva = kv_all[:]
                for p in range(4):
                    src = kva.rearrange("(r x) k -> x r k", r=NCORES)
                    nc.sync.dma_start(
                        ktq[p][:].rearrange("d (r k) -> d r k", r=NCORES),
                        src[p * P:(p + 1) * P, :, :])
                for t in range(NKT):
                    r, lt = divmod(t, 4)
                    base = r * 2 * D + D + lt * P
                    v3 = vpq[t][:].rearrange("p (h c) -> p h c", c=65)
                    nc.sync.dma_start(
                        v3[:, :, 0:64],
                        kva[base:base + P, :].rearrange("p (h c) -> p h c",
                                                        c=64))
                    nc.gpsimd.memset(v3[:, :, 64:65], 1.0)

                # ---- attention: pair-major, ctx accumulates in PSUM
                with (
                    tc.tile_pool(name="st", bufs=2, space="PSUM") as stp,
                    tc.tile_pool(name="ctxps", bufs=2, space="PSUM") as cpp,
                ):
                    def normalize(p, ctx01):
                        h0, h1 = 2 * p, 2 * p + 1
                        rs = nrmp.tile([1, 2 * SLICE], f32, tag="rs")
                        nc.vector.tensor_copy(rs[:], ctx01[64:65, :])
                        rc = nrmp.tile([1, 2 * SLICE], f32, tag="rc")
                        scr = nrmp.tile([1, 2 * SLICE], f32, tag="scr")
                        nc.vector.reciprocal_approx_accurate(rc[:], rs[:], scr[:])
                        bc = nrmp.tile([DK, 2 * SLICE], f32, tag="bc")
                        nc.gpsimd.partition_broadcast(bc[:], rc[0:1, :])
                        nc.vector.tensor_mul(ctxT[h0][:], ctx01[0:DK, 0:SLICE],
                                             bc[:, 0:SLICE])
                        nc.vector.tensor_mul(ctxT[h1][:], ctx01[0:DK, SLICE:],
                                             bc[:, SLICE:])

                    for p in range(4):
                        h0, h1 = 2 * p, 2 * p + 1
                        ctx01 = cpp.tile([65, 2 * SLICE], f32, tag="ctx")
                        esq = []

                        def emit_s(t):
                            stt = stp.tile([P, 2 * SLICE], f32, tag="st")
                            nc.tensor.matmul(
                                stt[:, 0:SLICE],
                                ktq[p][0:DK, t * P:(t + 1) * P],
                                qtS[p][0:DK, :], start=True, stop=True)
                            nc.tensor.matmul(
                                stt[:, SLICE:],
                                ktq[p][DK:P, t * P:(t + 1) * P],
                                qtS[p][DK:P, :], start=True, stop=True)
                            es = esp.tile([P, 2 * SLICE], f16, tag="es")
                            if _EXPA[t]:
                                nc.scalar.activation(es[:], stt[:], AF.Exp,
                                                     scale=ACT_SCALE)
                            else:
                                nc.vector.tensor_scalar(
                                    es[:].bitcast(u16), stt[:], EXP_B, 0.0,
                                    op0=ALU.add, op1=ALU.max)
                            esq.append(es)

                        def emit_c(t):
                            es = esq.pop(0)
                            nc.tensor.matmul(
                                ctx01[:, 0:SLICE],
                                vpq[t][:, h0 * 65:(h0 + 1) * 65],
                                es[:, 0:SLICE],
                                start=(t == 0), stop=(t == NKT - 1))
                            nc.tensor.matmul(
                                ctx01[:, SLICE:],
                                vpq[t][:, h1 * 65:(h1 + 1) * 65],
                                es[:, SLICE:],
                                start=(t == 0), stop=(t == NKT - 1))

                        emit_s(0)
                        for t in range(NKT):
                            if t + 1 < NKT:
                                emit_s(t + 1)
                            emit_c(t)
                        normalize(p, ctx01)

                # ---- phase D: out proj + residual + LayerNorm
                with (
                    tc.tile_pool(name="psD", bufs=4, space="PSUM") as psD,
                    tc.tile_pool(name="ln", bufs=2) as lnp,
                ):
                    for qs in range(NQS):
                        op = psD.tile([P, D], f32, tag="psD")
                        for h in range(H):
                            nc.tensor.matmul(
                                op[:], ctxT[h][:, qs * P:(qs + 1) * P],
                                woS[h][:], start=(h == 0), stop=(h == H - 1))
                        xt_ = lnp.tile([P, D], f32, tag="xres")
                        nc.sync.dma_start(xt_[:], xs[qs * P:(qs + 1) * P, :])
                        t_ = lnp.tile([P, D], f32, tag="t")
                        nc.vector.tensor_add(t_[:], op[:], xt_[:])
                        if has_bo:
                            nc.vector.tensor_add(t_[:], t_[:], bo_b[:])
                        s1 = lnp.tile([P, 1], f32, tag="s1")
                        nc.vector.reduce_sum(s1[:], t_[:],
                                             axis=mybir.AxisListType.X)
                        negmu = lnp.tile([P, 1], f32, tag="negmu")
                        nc.vector.tensor_scalar_mul(negmu[:], s1[:], -1.0 / D)
                        tcen = lnp.tile([P, D], f32, tag="tcen")
                        nc.vector.tensor_scalar_add(tcen[:], t_[:], negmu[:])
                        sq = lnp.tile([P, D], f32, tag="sq")
                        v1 = lnp.tile([P, 1], f32, tag="v1")
                        nc.scalar.activation(sq[:], tcen[:], AF.Square,
                                             accum_out=v1[:])
                        std = lnp.tile([P, 1], f32, tag="std")
                        nc.scalar.activation(std[:], v1[:], AF.Sqrt,
                                             bias=eps_t[:], scale=1.0 / D)
                        rstd = lnp.tile([P, 1], f32, tag="rstd")
                        nc.vector.reciprocal(rstd[:], std[:])
                        out_t = lnp.tile([P, D], f32, tag="out_t")
                        nc.vector.tensor_scalar_mul(out_t[:], tcen[:], rstd[:])
                        if has_gamma:
                            nc.vector.tensor_mul(out_t[:], out_t[:], gamma_b[:])
                        if has_beta:
                            nc.vector.tensor_add(out_t[:], out_t[:], beta_b[:])
                        nc.sync.dma_start(y[qs * P:(qs + 1) * P, :], out_t[:])

        if bench_reps and _NO_AG:
            with tc.For_i(0, bench_reps, 1):
                emit_body(kv_alls[0])
        else:
            for i in range(max(1, bench_reps)):
                emit_body(kv_alls[i])
    nc.compile()
    return nc


_NC_CACHE: dict = {}


def _get_nc(flags, bench_reps: int = 0):
    key = (flags, bench_reps, _N_ACT, _NO_AG)
    if key not in _NC_CACHE:
        _NC_CACHE[key] = _build_nc(*flags, bench_reps=bench_reps)
    return _NC_CACHE[key]


def _prep_inputs(inputs):
    x = np.ascontiguousarray(np.asarray(inputs["x"], dtype=np.float32))
    Wq = np.asarray(inputs["Wq"], dtype=np.float32)
    Wk = np.asarray(inputs["Wk"], dtype=np.float32)
    Wv = np.asarray(inputs["Wv"], dtype=np.float32)
    Wo = np.asarray(inputs["Wo"], dtype=np.float32)
    bq = np.asarray(inputs["bq"], dtype=np.float32)
    bk = np.asarray(inputs["bk"], dtype=np.float32)
    bv = np.asarray(inputs["bv"], dtype=np.float32)
    bo = np.asarray(inputs["bo"], dtype=np.float32)
    gamma = np.asarray(inputs["gamma"], dtype=np.float32)
    beta = np.asarray(inputs["beta"], dtype=np.float32)

    has_bias = bool(np.any(bq) or np.any(bk) or np.any(bv))
    has_bo = bool(np.any(bo))
    has_gamma = bool(np.any(gamma != 1.0))
    has_beta = bool(np.any(beta))
    flags = (has_bias, has_bo, has_gamma, has_beta)

    cast = lambda a: a.astype(np.float16)
    xT = np.concatenate([x.T, np.ones((1, S), np.float32)], axis=0)
    xT = cast(xT)
    wq_e = cast(np.concatenate([Wq, bq[None, :]], axis=0) * np.float32(SCPRE))
    wk_e = cast(np.concatenate([Wk, bk[None, :]], axis=0))
    wv_e = cast(np.concatenate([Wv, bv[None, :]], axis=0))
    wo_r = _round_f32r(Wo)

    shared = {
        "wq": wq_e, "wk": wk_e, "wv": wv_e, "wo": wo_r,
        "bo": bo.reshape(1, D), "gamma": gamma.reshape(1, D),
        "beta": beta.reshape(1, D),
    }
    in_maps = []
    for i in range(NCORES):
        m = dict(shared)
        m["xq"] = np.ascontiguousarray(xT[:, i * SLICE:(i + 1) * SLICE])
        m["x_slice"] = np.ascontiguousarray(x[i * SLICE:(i + 1) * SLICE, :])
        in_maps.append(m)
    return flags, in_maps


def _run(inputs, trace=False, **kw):
    flags, in_maps = _prep_inputs(inputs)
    nc = _get_nc(flags)
    res = run_bass_kernel_spmd(nc, in_maps, core_ids=list(range(NCORES)),
                               trace=trace, **kw)
    out = np.concatenate([res.results[i]["y"] for i in range(NCORES)], axis=0)
    return out, res


def kernel(**inputs) -> np.ndarray:
    out, _ = _run(inputs, trace=False)
    return out


# revision 14
# speedup vs baseline: 1.0002x; 1.0002x over previous
"""Trainium2 Bass kernel for nn_Attention_15109694948028.

Single-layer attention block: QKV proj -> 8-head SDPA (S=4096, d_k=64)
-> out proj -> residual -> LayerNorm.  fp32 I/O.

Sharding: sequence-parallel across 8 NeuronCores with a K/V AllGather.
Core i owns query rows [i*512, (i+1)*512).  It computes K^T and V only
for its OWN 512 k-rows (1/8 of the work), pushes them to an internal
DRAM bounce buffer, and an 8-core AllGather (runs on TOPSP/SDMA --
free wrt the compute engines) distributes the full K^T/V.  Attention
is then pair-major: for each head-pair, all 32 k-tiles accumulate
ctx directly in PSUM (no SBUF ctx accumulator / no per-round adds).

exp is split between ScalarE (exact, activation Exp) and VectorE via a
one-instruction Schraudolph trick: Q is pre-scaled on the host by
184.664 = 0.125 * 1024/ln2 (0.125 = 1/sqrt(d_k)), so scores arrive in
PSUM already in "fp16-bit units"; DVE computes
   es_bits_u16 = max(scores + (15360 - 44.5), 0)
which, viewed as fp16, is exp(scores/184.664) to ~3% max / 1.8% mean
element error (softmax-averaging shrinks this ~10-30x in ctx).
The ScalarE half uses activation(Exp, scale=ln2/1024) and is exact.

All matmul operands are fp16 (same PE rate as bf16, 3 extra mantissa
bits); out-proj in float32r; residual/LayerNorm in fp32.

Benchmarking support: _build_nc(bench_reps=N) emits the whole body N
times unrolled (collectives cannot live inside For_i); test.py times
(T(reps_hi) - T(reps_lo)) / (reps_hi - reps_lo).
"""

import numpy as np

import concourse.bacc as bacc
import concourse.tile as tile
from concourse import mybir
from concourse.bass_utils import run_bass_kernel_spmd

f32 = mybir.dt.float32
f32r = mybir.dt.float32r
f16 = mybir.dt.float16
u16 = mybir.dt.uint16
AF = mybir.ActivationFunctionType
ALU = mybir.AluOpType

S = 4096
D = 512
H = 8
DK = 64
NCORES = 8
SLICE = S // NCORES          # 512 query rows per core
P = 128
NKT = S // P                 # 32 k-tiles
NQS = SLICE // P             # 4 q-subtiles
EPS = 1e-5

# Q pre-scale: folds 1/sqrt(d_k) and the fp16 exp-trick slope into Wq.
SCPRE = 0.125 * 1024.0 / np.log(2.0)        # 184.6635
ACT_SCALE = float(np.log(2.0) / 1024.0)     # exp(scores * this) on ScalarE
EXP_B = 15360.0 - 44.5                      # fp16 Schraudolph offset

# which k-tiles use ScalarE for exp (rest use the DVE trick)
_N_ACT = 32
_EXPA = [(t * _N_ACT) // 32 != ((t + 1) * _N_ACT) // 32 for t in range(32)]

# benchmarking ablation: skip the AllGather (ktq/vpq read the garbage
# kv_all directly) so the body is control-flow-free and can sit inside
# tc.For_i for precise differential timing.  The graded path always
# runs with the collective.
_NO_AG = False

# timing ablations (bench only): "full", "nokv" (skip ktq/vpq loads),
# "noexp" (skip exp -> ctx reads stale es; pure PE pace),
# "allact"/"alldve" (exp on one engine only), "nophd" (skip phase D)
_ABL = "full"


def _round_f32r(a: np.ndarray) -> np.ndarray:
    b = np.ascontiguousarray(a, dtype=np.float32).view(np.uint32)
    lsb = (b >> np.uint32(12)) & np.uint32(1)
    return ((b + np.uint32(0x7FF) + lsb) & np.uint32(0xFFFFF000)).view(np.float32)


def _build_nc(has_bias: bool, has_bo: bool, has_gamma: bool, has_beta: bool,
              bench_reps: int = 0):
    nc = bacc.Bacc("TRN2", target_bir_lowering=False, debug=False)

    xq = nc.dram_tensor("xq", [D + 1, SLICE], f16, kind="ExternalInput")
    wq = nc.dram_tensor("wq", [D + 1, D], f16, kind="ExternalInput")
    wk = nc.dram_tensor("wk", [D + 1, D], f16, kind="ExternalInput")
    wv = nc.dram_tensor("wv", [D + 1, D], f16, kind="ExternalInput")
    wo = nc.dram_tensor("wo", [D, D], f32r, kind="ExternalInput")
    xs = nc.dram_tensor("x_slice", [SLICE, D], f32, kind="ExternalInput")
    bo = nc.dram_tensor("bo", [1, D], f32, kind="ExternalInput")
    gamma = nc.dram_tensor("gamma", [1, D], f32, kind="ExternalInput")
    beta = nc.dram_tensor("beta", [1, D], f32, kind="ExternalInput")
    y = nc.dram_tensor("y", [SLICE, D], f32, kind="ExternalOutput")

    # AllGather landing buffer: rank r rows [r*1024, r*1024+512) = K^T
    # slice (dims x k_local), rows [r*1024+512, (r+1)*1024) = V slice
    # (k_local x dims).  One Shared tensor per rep (collectives want
    # distinct known buffers).
    kv_alls = [
        nc.dram_tensor(f"kv_all{i}", [NCORES * 2 * D, 520], f16,
                       kind="Internal", addr_space="Shared")
        for i in range(max(1, bench_reps))
    ]

    with tile.TileContext(nc) as tc:
        def emit_body(kv_all):
            with (
                tc.tile_pool(name="dram", bufs=1, space="DRAM") as drp,
                tc.tile_pool(name="consts", bufs=1) as cp,
                tc.tile_pool(name="xqp", bufs=1) as xqp,
                tc.tile_pool(name="qt", bufs=1) as qtp,
                tc.tile_pool(name="ktq", bufs=1) as ktqp,
                tc.tile_pool(name="vpq", bufs=1) as vpqp,
                tc.tile_pool(name="es", bufs=3) as esp,
                tc.tile_pool(name="nrm", bufs=2) as nrmp,
                tc.tile_pool(name="ctxT", bufs=1) as ctp,
                tc.tile_pool(name="wts", bufs=1) as wtp,
            ):
                eps_t = cp.tile([P, 1], f32, tag="eps")
                nc.gpsimd.memset(eps_t[:], EPS)

                def bcast_row(dram_row, tag):
                    r = cp.tile([1, D], f32, tag=f"{tag}_row", name=f"{tag}_row")
                    nc.sync.dma_start(r[:], dram_row)
                    b = cp.tile([P, D], f32, tag=f"{tag}_b", name=f"{tag}_b")
                    nc.gpsimd.partition_broadcast(b[:], r[0:1, :])
                    return b

                bo_b = bcast_row(bo[:], "bo") if has_bo else None
                gamma_b = bcast_row(gamma[:], "gamma") if has_gamma else None
                beta_b = bcast_row(beta[:], "beta") if has_beta else None

                xqS = [xqp.tile([P, SLICE], f16, tag=f"xq{e}", name=f"xq{e}")
                       for e in range(4)]
                for e in range(4):
                    nc.sync.dma_start(xqS[e][:], xq[e * P:(e + 1) * P, :])
                if has_bias:
                    xq_ones = xqp.tile([1, SLICE], f16, tag="xq_ones")
                    nc.sync.dma_start(xq_ones[:], xq[D:D + 1, :])

                wkS = [wtp.tile([P, D], f16, tag=f"wk{e}", name=f"wk{e}")
                       for e in range(4)]
                wvS = [wtp.tile([P, D], f16, tag=f"wv{e}", name=f"wv{e}")
                       for e in range(4)]
                wqS = [wtp.tile([P, D], f16, tag=f"wq{e}", name=f"wq{e}")
                       for e in range(4)]
                for e in range(4):
                    nc.sync.dma_start(wkS[e][:], wk[e * P:(e + 1) * P, :])
                    nc.sync.dma_start(wvS[e][:], wv[e * P:(e + 1) * P, :])
                    nc.sync.dma_start(wqS[e][:], wq[e * P:(e + 1) * P, :])
                if has_bias:
                    wkb = wtp.tile([1, D], f16, tag="wkb")
                    wvb = wtp.tile([1, D], f16, tag="wvb")
                    wqb = wtp.tile([1, D], f16, tag="wqb")
                    nc.sync.dma_start(wkb[:], wk[D:D + 1, :])
                    nc.sync.dma_start(wvb[:], wv[D:D + 1, :])
                    nc.sync.dma_start(wqb[:], wq[D:D + 1, :])
                woS = [wtp.tile([DK, D], f32r, tag=f"wo{h}", name=f"wo{h}")
                       for h in range(H)]
                for h in range(H):
                    nc.sync.dma_start(woS[h][:], wo[h * DK:(h + 1) * DK, :])

                qtS = [qtp.tile([P, SLICE], f16, tag=f"qt{p}", name=f"qt{p}")
                       for p in range(4)]
                ktq = [ktqp.tile([P, S], f16, tag=f"ktq{p}", name=f"ktq{p}")
                       for p in range(4)]
                vpq = [vpqp.tile([P, H * 65], f16, tag=f"vpq{t}",
                                 name=f"vpq{t}") for t in range(NKT)]
                ctxT = [ctp.tile([DK, SLICE], f32r, tag=f"ctxT{h}",
                                 name=f"ctxT{h}") for h in range(H)]

                kv_loc = drp.tile([2 * D, 520], f16, tag="kv_loc")

                # ---- phase B0: local K^T and V slices -> DRAM -> AllGather
                with (
                    tc.tile_pool(name="psB", bufs=2, space="PSUM") as psB,
                    tc.tile_pool(name="stage", bufs=1) as stg,
                ):
                    for p in range(4):
                        ps = psB.tile([P, SLICE], f32, tag="psB")
                        for e in range(4):
                            nc.tensor.matmul(
                                ps[:], wkS[e][:, p * P:(p + 1) * P], xqS[e][:],
                                start=(e == 0), stop=(e == 3 and not has_bias))
                        if has_bias:
                            nc.tensor.matmul(ps[:], wkb[0:1, p * P:(p + 1) * P],
                                             xq_ones[:], start=False, stop=True)
                        kt_s = stg.tile([P, SLICE], f16, tag=f"kts{p}",
                                        name=f"kts{p}")
                        nc.scalar.copy(kt_s[:], ps[:])
                        nc.sync.dma_start(kv_loc[p * P:(p + 1) * P, 0:SLICE], kt_s[:])
                    for t in range(4):
                        ps = psB.tile([P, D], f32, tag="psB")
                        for e in range(4):
                            nc.tensor.matmul(
                                ps[:], xqS[e][:, t * P:(t + 1) * P], wvS[e][:],
                                start=(e == 0), stop=(e == 3 and not has_bias))
                        if has_bias:
                            nc.tensor.matmul(
                                ps[:], xq_ones[0:1, t * P:(t + 1) * P], wvb[:],
                                start=False, stop=True)
                        v_s = stg.tile([P, 520], f16, tag=f"vts{t}",
                                       name=f"vts{t}")
                        v3s = v_s[:].rearrange("p (h c) -> p h c", c=65)
                        nc.gpsimd.memset(v3s[:, :, 64:65], 1.0)
                        nc.vector.tensor_copy(
                            v3s[:, :, 0:64],
                            ps[:].rearrange("p (h c) -> p h c", c=64))
                        nc.sync.dma_start(kv_loc[D + t * P:D + (t + 1) * P, :],
                                          v_s[:])

                    if not _NO_AG:
                        nc.gpsimd.collective_compute(
                            "AllGather", ALU.bypass,
                            replica_groups=[list(range(NCORES))],
                            ins=[kv_loc[:]], outs=[kv_all[:]],
                        )

                    # ---- phase A: Q^T projection (overlaps the AllGather)
                    for p in range(4):
                        ps = psB.tile([P, SLICE], f32, tag="psB")
                        for e in range(4):
                            nc.tensor.matmul(
                                ps[:], wqS[e][:, p * P:(p + 1) * P], xqS[e][:],
                                start=(e == 0), stop=(e == 3 and not has_bias))
                        if has_bias:
                            nc.tensor.matmul(ps[:], wqb[0:1, p * P:(p + 1) * P],
                                             xq_ones[:], start=False, stop=True)
                        nc.vector.tensor_copy(qtS[p][:], ps[:])

                # ---- gathered K^T / V -> SBUF working layout
                kva = kv_all[:]
                if _ABL != "nokv":
                    ksrc = kva[:, 0:SLICE].rearrange("(r x) k -> x r k",
                                                     r=NCORES)

                    def dma_ktq(p):
                        nc.sync.dma_start(
                            ktq[p][:].rearrange("d (r k) -> d r k", r=NCORES),
                            ksrc[p * P:(p + 1) * P, :, :])

                    def dma_vpq(t):
                        r, lt = divmod(t, 4)
                        base = r * 2 * D + D + lt * P
                        nc.sync.dma_start(vpq[t][:], kva[base:base + P, :])

                    dma_ktq(0)
                    for t in range(16):
                        dma_vpq(t)
                    dma_ktq(1)
                    for t in range(16, NKT):
                        dma_vpq(t)
                    dma_ktq(2)
                    dma_ktq(3)

                # ---- attention: pair-major, ctx accumulates in PSUM
                with (
                    tc.tile_pool(name="st", bufs=2, space="PSUM") as stp,
                    tc.tile_pool(name="ctxps", bufs=2, space="PSUM") as cpp,
                ):
                    def normalize(p, ctx01):
                        h0, h1 = 2 * p, 2 * p + 1
                        rs = nrmp.tile([1, 2 * SLICE], f32, tag="rs")
                        nc.vector.tensor_copy(rs[:], ctx01[64:65, :])
                        rc = nrmp.tile([1, 2 * SLICE], f32, tag="rc")
                        scr = nrmp.tile([1, 2 * SLICE], f32, tag="scr")
                        nc.vector.reciprocal_approx_accurate(rc[:], rs[:], scr[:])
                        bc = nrmp.tile([DK, 2 * SLICE], f32, tag="bc")
                        nc.gpsimd.partition_broadcast(bc[:], rc[0:1, :])
                        nc.vector.tensor_mul(ctxT[h0][:], ctx01[0:DK, 0:SLICE],
                                             bc[:, 0:SLICE])
                        nc.vector.tensor_mul(ctxT[h1][:], ctx01[0:DK, SLICE:],
                                             bc[:, SLICE:])

                    es0 = None
                    if _ABL == "noexp":
                        es0 = esp.tile([P, 2 * SLICE], f16, tag="es0",
                                       name="es0")
                        nc.gpsimd.memset(es0[:], 0.25)
                    for p in range(4):
                        h0, h1 = 2 * p, 2 * p + 1
                        ctx01 = cpp.tile([65, 2 * SLICE], f32, tag="ctx")
                        esq = []

                        def emit_s(t):
                            stt = stp.tile([P, 2 * SLICE], f32, tag="st")
                            nc.tensor.matmul(
                                stt[:, 0:SLICE],
                                ktq[p][0:DK, t * P:(t + 1) * P],
                                qtS[p][0:DK, :], start=True, stop=True)
                            nc.tensor.matmul(
                                stt[:, SLICE:],
                                ktq[p][DK:P, t * P:(t + 1) * P],
                                qtS[p][DK:P, :], start=True, stop=True)
                            if _ABL == "noexp":
                                esq.append(es0)
                                return
                            es = esp.tile([P, 2 * SLICE], f16, tag="es")
                            use_act = _EXPA[t] if _ABL not in ("allact", "alldve") \
                                else (_ABL == "allact")
                            if use_act:
                                nc.scalar.activation(es[:], stt[:], AF.Exp,
                                                     scale=ACT_SCALE)
                            else:
                                nc.vector.tensor_scalar(
                                    es[:].bitcast(u16), stt[:], EXP_B, 0.0,
                                    op0=ALU.add, op1=ALU.max)
                            esq.append(es)

                        def emit_c(t):
                            es = esq.pop(0)
                            nc.tensor.matmul(
                                ctx01[:, 0:SLICE],
                                vpq[t][:, h0 * 65:(h0 + 1) * 65],
                                es[:, 0:SLICE],
                                start=(t == 0), stop=(t == NKT - 1))
                            nc.tensor.matmul(
                                ctx01[:, SLICE:],
                                vpq[t][:, h1 * 65:(h1 + 1) * 65],
                                es[:, SLICE:],
                                start=(t == 0), stop=(t == NKT - 1))

                        emit_s(0)
                        for t in range(NKT):
                            if t + 1 < NKT:
                                emit_s(t + 1)
                            emit_c(t)
                        normalize(p, ctx01)

                # ---- phase D: out proj + residual + LayerNorm
                with (
                    tc.tile_pool(name="psD", bufs=4, space="PSUM") as psD,
                    tc.tile_pool(name="ln", bufs=2) as lnp,
                ):
                    for qs in range(NQS if _ABL != "nophd" else 0):
                        op = psD.tile([P, D], f32, tag="psD")
                        for h in range(H):
                            nc.tensor.matmul(
                                op[:], ctxT[h][:, qs * P:(qs + 1) * P],
                                woS[h][:], start=(h == 0), stop=(h == H - 1))
                        xt_ = lnp.tile([P, D], f32, tag="xres")
                        nc.sync.dma_start(xt_[:], xs[qs * P:(qs + 1) * P, :])
                        t_ = lnp.tile([P, D], f32, tag="t")
                        nc.vector.tensor_add(t_[:], op[:], xt_[:])
                        if has_bo:
                            nc.vector.tensor_add(t_[:], t_[:], bo_b[:])
                        s1 = lnp.tile([P, 1], f32, tag="s1")
                        nc.vector.reduce_sum(s1[:], t_[:],
                                             axis=mybir.AxisListType.X)
                        negmu = lnp.tile([P, 1], f32, tag="negmu")
                        nc.vector.tensor_scalar_mul(negmu[:], s1[:], -1.0 / D)
                        tcen = lnp.tile([P, D], f32, tag="tcen")
                        nc.vector.tensor_scalar_add(tcen[:], t_[:], negmu[:])
                        sq = lnp.tile([P, D], f32, tag="sq")
                        v1 = lnp.tile([P, 1], f32, tag="v1")
                        nc.scalar.activation(sq[:], tcen[:], AF.Square,
                                             accum_out=v1[:])
                        std = lnp.tile([P, 1], f32, tag="std")
                        nc.scalar.activation(std[:], v1[:], AF.Sqrt,
                                             bias=eps_t[:], scale=1.0 / D)
                        rstd = lnp.tile([P, 1], f32, tag="rstd")
                        nc.vector.reciprocal(rstd[:], std[:])
                        out_t = lnp.tile([P, D], f32, tag="out_t")
                        nc.vector.tensor_scalar_mul(out_t[:], tcen[:], rstd[:])
                        if has_gamma:
                            nc.vector.tensor_mul(out_t[:], out_t[:], gamma_b[:])
                        if has_beta:
                            nc.vector.tensor_add(out_t[:], out_t[:], beta_b[:])
                        nc.sync.dma_start(y[qs * P:(qs + 1) * P, :], out_t[:])

        if bench_reps and _NO_AG:
            with tc.For_i(0, bench_reps, 1):
                emit_body(kv_alls[0])
        else:
            for i in range(max(1, bench_reps)):
                emit_body(kv_alls[i])
    nc.compile()
    return nc


_NC_CACHE: dict = {}


def _get_nc(flags, bench_reps: int = 0):
    key = (flags, bench_reps, _N_ACT, _NO_AG, _ABL)
    if key not in _NC_CACHE:
        _NC_CACHE[key] = _build_nc(*flags, bench_reps=bench_reps)
    return _NC_CACHE[key]


def _prep_inputs(inputs):
    x = np.ascontiguousarray(np.asarray(inputs["x"], dtype=np.float32))
    Wq = np.asarray(inputs["Wq"], dtype=np.float32)
    Wk = np.asarray(inputs["Wk"], dtype=np.float32)
    Wv = np.asarray(inputs["Wv"], dtype=np.float32)
    Wo = np.asarray(inputs["Wo"], dtype=np.float32)
    bq = np.asarray(inputs["bq"], dtype=np.float32)
    bk = np.asarray(inputs["bk"], dtype=np.float32)
    bv = np.asarray(inputs["bv"], dtype=np.float32)
    bo = np.asarray(inputs["bo"], dtype=np.float32)
    gamma = np.asarray(inputs["gamma"], dtype=np.float32)
    beta = np.asarray(inputs["beta"], dtype=np.float32)

    has_bias = bool(np.any(bq) or np.any(bk) or np.any(bv))
    has_bo = bool(np.any(bo))
    has_gamma = bool(np.any(gamma != 1.0))
    has_beta = bool(np.any(beta))
    flags = (has_bias, has_bo, has_gamma, has_beta)

    cast = lambda a: a.astype(np.float16)
    xT = np.concatenate([x.T, np.ones((1, S), np.float32)], axis=0)
    xT = cast(xT)
    wq_e = cast(np.concatenate([Wq, bq[None, :]], axis=0) * np.float32(SCPRE))
    wk_e = cast(np.concatenate([Wk, bk[None, :]], axis=0))
    wv_e = cast(np.concatenate([Wv, bv[None, :]], axis=0))
    wo_r = _round_f32r(Wo)

    shared = {
        "wq": wq_e, "wk": wk_e, "wv": wv_e, "wo": wo_r,
        "bo": bo.reshape(1, D), "gamma": gamma.reshape(1, D),
        "beta": beta.reshape(1, D),
    }
    in_maps = []
    for i in range(NCORES):
        m = dict(shared)
        m["xq"] = np.ascontiguousarray(xT[:, i * SLICE:(i + 1) * SLICE])
        m["x_slice"] = np.ascontiguousarray(x[i * SLICE:(i + 1) * SLICE, :])
        in_maps.append(m)
    return flags, in_maps


def _run(inputs, trace=False, **kw):
    flags, in_maps = _prep_inputs(inputs)
    nc = _get_nc(flags)
    res = run_bass_kernel_spmd(nc, in_maps, core_ids=list(range(NCORES)),
                               trace=trace, **kw)
    out = np.concatenate([res.results[i]["y"] for i in range(NCORES)], axis=0)
    return out, res


def kernel(**inputs) -> np.ndarray:
    out, _ = _run(inputs, trace=False)
    return out


# revision 20
# speedup vs baseline: 1.4129x; 1.4127x over previous
"""Trainium2 Bass kernel for nn_Attention_15109694948028.

Single-layer attention block: QKV proj -> 8-head SDPA (S=4096, d_k=64)
-> out proj -> residual -> LayerNorm.  fp32 I/O.

Sharding: sequence-parallel across 8 NeuronCores with a K/V AllGather.
Core i owns query rows [i*512, (i+1)*512).  It computes K^T and V only
for its OWN 512 k-rows (1/8 of the work), pushes them to an internal
DRAM bounce buffer, and an 8-core AllGather (runs on TOPSP/SDMA --
free wrt the compute engines) distributes the full K^T/V.  Attention
is then pair-major: for each head-pair, all 32 k-tiles accumulate
ctx directly in PSUM (no SBUF ctx accumulator / no per-round adds).

exp is split between ScalarE (exact, activation Exp) and VectorE via a
one-instruction Schraudolph trick: Q is pre-scaled on the host by
184.664 = 0.125 * 1024/ln2 (0.125 = 1/sqrt(d_k)), so scores arrive in
PSUM already in "fp16-bit units"; DVE computes
   es_bits_u16 = max(scores + (15360 - 44.5), 0)
which, viewed as fp16, is exp(scores/184.664) to ~3% max / 1.8% mean
element error (softmax-averaging shrinks this ~10-30x in ctx).
The ScalarE half uses activation(Exp, scale=ln2/1024) and is exact.

All matmul operands are fp16 (same PE rate as bf16, 3 extra mantissa
bits); out-proj in float32r; residual/LayerNorm in fp32.

Benchmarking support: _build_nc(bench_reps=N) emits the whole body N
times unrolled (collectives cannot live inside For_i); test.py times
(T(reps_hi) - T(reps_lo)) / (reps_hi - reps_lo).
"""

import numpy as np

import concourse.bacc as bacc
import concourse.tile as tile
from concourse import mybir
from concourse.bass_utils import run_bass_kernel_spmd

f32 = mybir.dt.float32
f32r = mybir.dt.float32r
f16 = mybir.dt.float16
u16 = mybir.dt.uint16
u8 = mybir.dt.uint8
f8 = mybir.dt.float8e4
AF = mybir.ActivationFunctionType
ALU = mybir.AluOpType

S = 4096
D = 512
H = 8
DK = 64
NCORES = 8
SLICE = S // NCORES          # 512 query rows per core
P = 128
NKT = S // P                 # 32 k-tiles
NQS = SLICE // P             # 4 q-subtiles
EPS = 1e-5

# Q pre-scale: folds 1/sqrt(d_k) and the exp-trick slope into Wq.
# fp16 es: slope 1024/ln2 per logit; fp8e4m3 es: slope 8/ln2 per logit.
_CTX_FP8 = True
SCPRE16 = 0.125 * 1024.0 / np.log(2.0)      # 184.6635
SCPRE8 = 0.125 * 8.0 / np.log(2.0)          # 1.442695
SCPRE = SCPRE8 if _CTX_FP8 else SCPRE16
ACT_SCALE = float(np.log(2.0) / (8.0 if _CTX_FP8 else 1024.0))
EXP_B = (56.0 - 16.0 - 0.3477) if _CTX_FP8 else (15360.0 - 44.5)
ACT_BIAS = float(-2.0 * np.log(2.0)) if _CTX_FP8 else 0.0

# which k-tiles use ScalarE for exp (rest use the DVE trick)
_N_ACT = 32
_EXPA = [(t * _N_ACT) // 32 != ((t + 1) * _N_ACT) // 32 for t in range(32)]

# benchmarking ablation: skip the AllGather (ktq/vpq read the garbage
# kv_all directly) so the body is control-flow-free and can sit inside
# tc.For_i for precise differential timing.  The graded path always
# runs with the collective.
_NO_AG = False

# timing ablations (bench only): "full", "nokv" (skip ktq/vpq loads),
# "noexp" (skip exp -> ctx reads stale es; pure PE pace),
# "allact"/"alldve" (exp on one engine only), "nophd" (skip phase D)
_ABL = "full"
_DBG = False


def _round_f32r(a: np.ndarray) -> np.ndarray:
    b = np.ascontiguousarray(a, dtype=np.float32).view(np.uint32)
    lsb = (b >> np.uint32(12)) & np.uint32(1)
    return ((b + np.uint32(0x7FF) + lsb) & np.uint32(0xFFFFF000)).view(np.float32)


def _build_nc(has_bias: bool, has_bo: bool, has_gamma: bool, has_beta: bool,
              bench_reps: int = 0):
    nc = bacc.Bacc("TRN2", target_bir_lowering=False, debug=False)

    xq = nc.dram_tensor("xq", [D + 1, SLICE], f16, kind="ExternalInput")
    wq = nc.dram_tensor("wq", [D + 1, D], f16, kind="ExternalInput")
    wk = nc.dram_tensor("wk", [D + 1, D], f16, kind="ExternalInput")
    wv = nc.dram_tensor("wv", [D + 1, D], f16, kind="ExternalInput")
    wo = nc.dram_tensor("wo", [D, D], f32r, kind="ExternalInput")
    xs = nc.dram_tensor("x_slice", [SLICE, D], f32, kind="ExternalInput")
    bo = nc.dram_tensor("bo", [1, D], f32, kind="ExternalInput")
    gamma = nc.dram_tensor("gamma", [1, D], f32, kind="ExternalInput")
    beta = nc.dram_tensor("beta", [1, D], f32, kind="ExternalInput")
    y = nc.dram_tensor("y", [SLICE, D], f32, kind="ExternalOutput")
    if _DBG:
        dbg_v = nc.dram_tensor("dbg_v", [P, 1280], f32, kind="ExternalOutput")
        dbg_es = nc.dram_tensor("dbg_es", [P, 4 * SLICE], f32,
                                kind="ExternalOutput")
        dbg_den = nc.dram_tensor("dbg_den", [1, 2 * SLICE], f32,
                                 kind="ExternalOutput")

    # AllGather landing buffer: rank r rows [r*1024, r*1024+512) = K^T
    # slice (dims x k_local), rows [r*1024+512, (r+1)*1024) = V slice
    # (k_local x dims).  One Shared tensor per rep (collectives want
    # distinct known buffers).
    if _CTX_FP8:
        kv_alls = [
            (nc.dram_tensor(f"kv_allk{i}", [NCORES * D, SLICE], f16,
                            kind="Internal", addr_space="Shared"),
             nc.dram_tensor(f"kv_allv{i}", [NCORES * D, 640], f8,
                            kind="Internal", addr_space="Shared"))
            for i in range(max(1, bench_reps))
        ]
    else:
        kv_alls = [
            nc.dram_tensor(f"kv_all{i}", [NCORES * 2 * D, 520], f16,
                           kind="Internal", addr_space="Shared")
            for i in range(max(1, bench_reps))
        ]

    with tile.TileContext(nc) as tc:
        def emit_body(kv_all):
            with (
                tc.tile_pool(name="dram", bufs=1, space="DRAM") as drp,
                tc.tile_pool(name="consts", bufs=1) as cp,
                tc.tile_pool(name="xqp", bufs=1) as xqp,
                tc.tile_pool(name="qt", bufs=1) as qtp,
                tc.tile_pool(name="ktq", bufs=1) as ktqp,
                tc.tile_pool(name="vpq", bufs=1) as vpqp,
                tc.tile_pool(name="es", bufs=3) as esp,
                tc.tile_pool(name="nrm", bufs=2) as nrmp,
                tc.tile_pool(name="ctxT", bufs=1) as ctp,
                tc.tile_pool(name="wts", bufs=1) as wtp,
            ):
                eps_t = cp.tile([P, 1], f32, tag="eps")
                nc.gpsimd.memset(eps_t[:], EPS)
                nb_t = None
                if _CTX_FP8:
                    nb_t = cp.tile([P, 1], f32, tag="nb")
                    nc.gpsimd.memset(nb_t[:], ACT_BIAS)

                def bcast_row(dram_row, tag):
                    r = cp.tile([1, D], f32, tag=f"{tag}_row", name=f"{tag}_row")
                    nc.sync.dma_start(r[:], dram_row)
                    b = cp.tile([P, D], f32, tag=f"{tag}_b", name=f"{tag}_b")
                    nc.gpsimd.partition_broadcast(b[:], r[0:1, :])
                    return b

                bo_b = bcast_row(bo[:], "bo") if has_bo else None
                gamma_b = bcast_row(gamma[:], "gamma") if has_gamma else None
                beta_b = bcast_row(beta[:], "beta") if has_beta else None

                xqS = [xqp.tile([P, SLICE], f16, tag=f"xq{e}", name=f"xq{e}")
                       for e in range(4)]
                for e in range(4):
                    nc.sync.dma_start(xqS[e][:], xq[e * P:(e + 1) * P, :])
                if has_bias:
                    xq_ones = xqp.tile([1, SLICE], f16, tag="xq_ones")
                    nc.sync.dma_start(xq_ones[:], xq[D:D + 1, :])

                wkS = [wtp.tile([P, D], f16, tag=f"wk{e}", name=f"wk{e}")
                       for e in range(4)]
                wvS = [wtp.tile([P, D], f16, tag=f"wv{e}", name=f"wv{e}")
                       for e in range(4)]
                wqS = [wtp.tile([P, D], f16, tag=f"wq{e}", name=f"wq{e}")
                       for e in range(4)]
                for e in range(4):
                    nc.sync.dma_start(wkS[e][:], wk[e * P:(e + 1) * P, :])
                    nc.sync.dma_start(wvS[e][:], wv[e * P:(e + 1) * P, :])
                    nc.sync.dma_start(wqS[e][:], wq[e * P:(e + 1) * P, :])
                if has_bias:
                    wkb = wtp.tile([1, D], f16, tag="wkb")
                    wvb = wtp.tile([1, D], f16, tag="wvb")
                    wqb = wtp.tile([1, D], f16, tag="wqb")
                    nc.sync.dma_start(wkb[:], wk[D:D + 1, :])
                    nc.sync.dma_start(wvb[:], wv[D:D + 1, :])
                    nc.sync.dma_start(wqb[:], wq[D:D + 1, :])
                woS = [wtp.tile([DK, D], f32r, tag=f"wo{h}", name=f"wo{h}")
                       for h in range(H)]
                for h in range(H):
                    nc.sync.dma_start(woS[h][:], wo[h * DK:(h + 1) * DK, :])

                qtS = [qtp.tile([P, SLICE], f16, tag=f"qt{p}", name=f"qt{p}")
                       for p in range(4)]
                ktq = [ktqp.tile([P, S], f16, tag=f"ktq{p}", name=f"ktq{p}")
                       for p in range(4)]
                if _CTX_FP8:
                    vpq = [vpqp.tile([P, 2 * 640], f8, tag=f"vpq{t2}",
                                     name=f"vpq{t2}") for t2 in range(NKT // 2)]
                else:
                    vpq = [vpqp.tile([P, H * 65], f16, tag=f"vpq{t}",
                                     name=f"vpq{t}") for t in range(NKT)]
                ctxT = [ctp.tile([DK, SLICE], f32r, tag=f"ctxT{h}",
                                 name=f"ctxT{h}") for h in range(H)]

                if _CTX_FP8:
                    kv_loc_k = drp.tile([D, SLICE], f16, tag="kv_loc_k")
                    kv_loc_v = drp.tile([D, 640], f8, tag="kv_loc_v")
                else:
                    kv_loc = drp.tile([2 * D, 520], f16, tag="kv_loc")

                # ---- phase B0: local K^T and V slices -> DRAM -> AllGather
                with (
                    tc.tile_pool(name="psB", bufs=2, space="PSUM") as psB,
                    tc.tile_pool(name="stage", bufs=1) as stg,
                ):
                    for p in range(4):
                        ps = psB.tile([P, SLICE], f32, tag="psB")
                        for e in range(4):
                            nc.tensor.matmul(
                                ps[:], wkS[e][:, p * P:(p + 1) * P], xqS[e][:],
                                start=(e == 0), stop=(e == 3 and not has_bias))
                        if has_bias:
                            nc.tensor.matmul(ps[:], wkb[0:1, p * P:(p + 1) * P],
                                             xq_ones[:], start=False, stop=True)
                        kt_s = stg.tile([P, SLICE], f16, tag=f"kts{p}",
                                        name=f"kts{p}")
                        nc.scalar.copy(kt_s[:], ps[:])
                        kdst = kv_loc_k if _CTX_FP8 else kv_loc
                        nc.sync.dma_start(kdst[p * P:(p + 1) * P, 0:SLICE], kt_s[:])
                    for t in range(4):
                        ps = psB.tile([P, D], f32, tag="psB")
                        for e in range(4):
                            nc.tensor.matmul(
                                ps[:], xqS[e][:, t * P:(t + 1) * P], wvS[e][:],
                                start=(e == 0), stop=(e == 3 and not has_bias))
                        if has_bias:
                            nc.tensor.matmul(
                                ps[:], xq_ones[0:1, t * P:(t + 1) * P], wvb[:],
                                start=False, stop=True)
                        if _CTX_FP8:
                            v_s = stg.tile([P, 640], f8, tag=f"vts{t}",
                                           name=f"vts{t}")
                            v3s = v_s[:].rearrange("p (h c) -> p h c", c=80)
                            nc.gpsimd.memset(v3s[:, :, 64:65], 1.0)
                            nc.vector.tensor_copy(
                                v3s[:, :, 0:64],
                                ps[:].rearrange("p (h c) -> p h c", c=64))
                            nc.sync.dma_start(
                                kv_loc_v[t * P:(t + 1) * P, :], v_s[:])
                        else:
                            v_s = stg.tile([P, 520], f16, tag=f"vts{t}",
                                           name=f"vts{t}")
                            v3s = v_s[:].rearrange("p (h c) -> p h c", c=65)
                            nc.gpsimd.memset(v3s[:, :, 64:65], 1.0)
                            nc.vector.tensor_copy(
                                v3s[:, :, 0:64],
                                ps[:].rearrange("p (h c) -> p h c", c=64))
                            nc.sync.dma_start(
                                kv_loc[D + t * P:D + (t + 1) * P, :], v_s[:])

                    if not _NO_AG:
                        if _CTX_FP8:
                            nc.gpsimd.collective_compute(
                                "AllGather", ALU.bypass,
                                replica_groups=[list(range(NCORES))],
                                ins=[kv_loc_k[:]], outs=[kv_all[0][:]],
                            )
                            nc.gpsimd.collective_compute(
                                "AllGather", ALU.bypass,
                                replica_groups=[list(range(NCORES))],
                                ins=[kv_loc_v[:]], outs=[kv_all[1][:]],
                            )
                        else:
                            nc.gpsimd.collective_compute(
                                "AllGather", ALU.bypass,
                                replica_groups=[list(range(NCORES))],
                                ins=[kv_loc[:]], outs=[kv_all[:]],
                            )

                    # ---- phase A: Q^T projection (overlaps the AllGather)
                    for p in range(4):
                        ps = psB.tile([P, SLICE], f32, tag="psB")
                        for e in range(4):
                            nc.tensor.matmul(
                                ps[:], wqS[e][:, p * P:(p + 1) * P], xqS[e][:],
                                start=(e == 0), stop=(e == 3 and not has_bias))
                        if has_bias:
                            nc.tensor.matmul(ps[:], wqb[0:1, p * P:(p + 1) * P],
                                             xq_ones[:], start=False, stop=True)
                        nc.vector.tensor_copy(qtS[p][:], ps[:])

                # ---- gathered K^T / V -> SBUF working layout
                if _ABL != "nokv":
                    if _CTX_FP8:
                        kvk, kvv = kv_all[0][:], kv_all[1][:]
                        ksrc = kvk.rearrange("(r x) k -> x r k", r=NCORES)

                        def dma_ktq(p):
                            nc.sync.dma_start(
                                ktq[p][:].rearrange("d (r k) -> d r k",
                                                    r=NCORES),
                                ksrc[p * P:(p + 1) * P, :, :])

                        def dma_vpq(t2):
                            for j in (0, 1):
                                t = 2 * t2 + j
                                r, lt = divmod(t, 4)
                                base = r * D + lt * P
                                nc.sync.dma_start(
                                    vpq[t2][:, j * 640:(j + 1) * 640],
                                    kvv[base:base + P, :])

                        dma_ktq(0)
                        for t2 in range(8):
                            dma_vpq(t2)
                        dma_ktq(1)
                        for t2 in range(8, NKT // 2):
                            dma_vpq(t2)
                        dma_ktq(2)
                        dma_ktq(3)
                    else:
                        kva = kv_all[:]
                        ksrc = kva[:, 0:SLICE].rearrange("(r x) k -> x r k",
                                                         r=NCORES)

                        def dma_ktq(p):
                            nc.sync.dma_start(
                                ktq[p][:].rearrange("d (r k) -> d r k",
                                                    r=NCORES),
                                ksrc[p * P:(p + 1) * P, :, :])

                        def dma_vpq(t):
                            r, lt = divmod(t, 4)
                            base = r * 2 * D + D + lt * P
                            nc.sync.dma_start(vpq[t][:], kva[base:base + P, :])

                        dma_ktq(0)
                        for t in range(16):
                            dma_vpq(t)
                        dma_ktq(1)
                        for t in range(16, NKT):
                            dma_vpq(t)
                        dma_ktq(2)
                        dma_ktq(3)

                # ---- attention: pair-major, ctx accumulates in PSUM
                stb = 3 if _ABL == "stb3" else 2
                ctb = 1 if _ABL == "stb3" else 2
                with (
                    tc.tile_pool(name="st", bufs=stb, space="PSUM") as stp,
                    tc.tile_pool(name="ctxps", bufs=ctb, space="PSUM") as cpp,
                    tc.tile_pool(name="esd", bufs=2) as esdp,
                ):
                    def normalize(p, ctx01):
                        h0, h1 = 2 * p, 2 * p + 1
                        rs = nrmp.tile([1, 2 * SLICE], f32, tag="rs")
                        nc.vector.tensor_copy(rs[:], ctx01[64:65, :])
                        if _DBG and p == 0:
                            nc.sync.dma_start(dbg_den[:], rs[:])
                        rc = nrmp.tile([1, 2 * SLICE], f32, tag="rc")
                        scr = nrmp.tile([1, 2 * SLICE], f32, tag="scr")
                        nc.vector.reciprocal_approx_accurate(rc[:], rs[:], scr[:])
                        bc = nrmp.tile([DK, 2 * SLICE], f32, tag="bc")
                        nc.gpsimd.partition_broadcast(bc[:], rc[0:1, :])
                        nc.vector.tensor_mul(ctxT[h0][:], ctx01[0:DK, 0:SLICE],
                                             bc[:, 0:SLICE])
                        nc.vector.tensor_mul(ctxT[h1][:], ctx01[0:DK, SLICE:],
                                             bc[:, SLICE:])

                    es0 = None
                    if _ABL == "noexp":
                        es0 = esp.tile([P, (4 if _CTX_FP8 else 2) * SLICE],
                                       f8 if _CTX_FP8 else f16, tag="es0",
                                       name="es0")
                        nc.gpsimd.memset(es0[:], 0.25)

                    def pair_fp8(p):
                        h0, h1 = 2 * p, 2 * p + 1
                        ctx01 = cpp.tile([65, 2 * SLICE], f32, tag="ctx")
                        es2q = []

                        def emit_s8(t, es2):
                            stt = stp.tile([P, 2 * SLICE], f32, tag="st")
                            nc.tensor.matmul(
                                stt[:, 0:SLICE],
                                ktq[p][0:DK, t * P:(t + 1) * P],
                                qtS[p][0:DK, :], start=True, stop=True)
                            nc.tensor.matmul(
                                stt[:, SLICE:],
                                ktq[p][DK:P, t * P:(t + 1) * P],
                                qtS[p][DK:P, :], start=True, stop=True)
                            if _ABL == "noexp":
                                return
                            half = es2[:, (t % 2) * 1024:(t % 2) * 1024 + 1024]
                            if _ABL == "splitiso":
                                use_act = (t % 8) < 6
                            elif _ABL == "alldve":
                                use_act = False
                            else:
                                use_act = True
                            if use_act:
                                nc.scalar.activation(half, stt[:], AF.Exp,
                                                     scale=ACT_SCALE,
                                                     bias=nb_t[:])
                            else:
                                nc.vector.tensor_scalar(
                                    half.bitcast(u8), stt[:], EXP_B, 0.0,
                                    op0=ALU.add, op1=ALU.max)

                        def emit_c8(t2):
                            es2 = es2q.pop(0)
                            e3 = (es0 if _ABL == "noexp" else es2)[:] \
                                .rearrange("p (o q) -> p o q", o=2)
                            v4 = vpq[t2][:].rearrange("p (o h c) -> p o h c",
                                                      o=2, c=80)
                            for hi, h in ((0, h0), (1, h1)):
                                for ch in range(2):
                                    q0 = hi * 512 + ch * 256
                                    nc.tensor.matmul(
                                        ctx01[0:65, q0:q0 + 256],
                                        v4[:, :, h, 0:65],
                                        e3[:, :, q0:q0 + 256],
                                        start=(t2 == 0), stop=(t2 == 15),
                                        perf_mode=mybir.MatmulPerfMode.DoubleRow)

                        for t2 in range(NKT // 2):
                            if _ABL == "noexp":
                                es2 = None
                            elif _ABL == "splitiso" and (2 * t2) % 8 >= 6:
                                es2 = esdp.tile([P, 2 * 2 * SLICE], f8,
                                                tag="esd")
                            else:
                                es2 = esp.tile([P, 2 * 2 * SLICE], f8,
                                               tag="es")
                            emit_s8(2 * t2, es2)
                            emit_s8(2 * t2 + 1, es2)
                            if _DBG and p == 0 and t2 == 0:
                                dv = nrmp.tile([P, 1280], f32, tag="dv")
                                nc.vector.tensor_copy(dv[:], vpq[0][:])
                                nc.sync.dma_start(dbg_v[:], dv[:])
                                de = nrmp.tile([P, 4 * SLICE], f32, tag="de")
                                nc.vector.tensor_copy(de[:], es2[:])
                                nc.sync.dma_start(dbg_es[:], de[:])
                            es2q.append(es2)
                            if t2 >= 1:
                                emit_c8(t2 - 1)
                        emit_c8(NKT // 2 - 1)
                        normalize(p, ctx01)

                    for p in range(4):
                        if _CTX_FP8:
                            pair_fp8(p)
                            continue
                        h0, h1 = 2 * p, 2 * p + 1
                        ctx01 = cpp.tile([65, 2 * SLICE], f32, tag="ctx")
                        esq = []

                        def emit_s(t):
                            stt = stp.tile([P, 2 * SLICE], f32, tag="st")
                            nc.tensor.matmul(
                                stt[:, 0:SLICE],
                                ktq[p][0:DK, t * P:(t + 1) * P],
                                qtS[p][0:DK, :], start=True, stop=True)
                            nc.tensor.matmul(
                                stt[:, SLICE:],
                                ktq[p][DK:P, t * P:(t + 1) * P],
                                qtS[p][DK:P, :], start=True, stop=True)
                            if _ABL == "noexp":
                                esq.append(es0)
                                return
                            if _ABL == "splitiso":
                                use_act = (t % 8) < 6
                            elif _ABL == "allact" or _ABL == "stb3":
                                use_act = True
                            elif _ABL == "alldve":
                                use_act = False
                            else:
                                use_act = _EXPA[t]
                            pool = esp if use_act else esdp
                            es = pool.tile([P, 2 * SLICE], f16, tag="es")
                            if use_act:
                                nc.scalar.activation(es[:], stt[:], AF.Exp,
                                                     scale=ACT_SCALE)
                            else:
                                nc.vector.tensor_scalar(
                                    es[:].bitcast(u16), stt[:], EXP_B, 0.0,
                                    op0=ALU.add, op1=ALU.max)
                            esq.append(es)

                        def emit_c(t):
                            es = esq.pop(0)
                            nc.tensor.matmul(
                                ctx01[:, 0:SLICE],
                                vpq[t][:, h0 * 65:(h0 + 1) * 65],
                                es[:, 0:SLICE],
                                start=(t == 0), stop=(t == NKT - 1))
                            nc.tensor.matmul(
                                ctx01[:, SLICE:],
                                vpq[t][:, h1 * 65:(h1 + 1) * 65],
                                es[:, SLICE:],
                                start=(t == 0), stop=(t == NKT - 1))

                        emit_s(0)
                        for t in range(NKT):
                            if t + 1 < NKT:
                                emit_s(t + 1)
                            emit_c(t)
                        normalize(p, ctx01)

                # ---- phase D: out proj + residual + LayerNorm
                with (
                    tc.tile_pool(name="psD", bufs=4, space="PSUM") as psD,
                    tc.tile_pool(name="ln", bufs=2) as lnp,
                ):
                    for qs in range(NQS if _ABL != "nophd" else 0):
                        op = psD.tile([P, D], f32, tag="psD")
                        for h in range(H):
                            nc.tensor.matmul(
                                op[:], ctxT[h][:, qs * P:(qs + 1) * P],
                                woS[h][:], start=(h == 0), stop=(h == H - 1))
                        xt_ = lnp.tile([P, D], f32, tag="xres")
                        nc.sync.dma_start(xt_[:], xs[qs * P:(qs + 1) * P, :])
                        t_ = lnp.tile([P, D], f32, tag="t")
                        nc.vector.tensor_add(t_[:], op[:], xt_[:])
                        if has_bo:
                            nc.vector.tensor_add(t_[:], t_[:], bo_b[:])
                        s1 = lnp.tile([P, 1], f32, tag="s1")
                        nc.vector.reduce_sum(s1[:], t_[:],
                                             axis=mybir.AxisListType.X)
                        negmu = lnp.tile([P, 1], f32, tag="negmu")
                        nc.vector.tensor_scalar_mul(negmu[:], s1[:], -1.0 / D)
                        tcen = lnp.tile([P, D], f32, tag="tcen")
                        nc.vector.tensor_scalar_add(tcen[:], t_[:], negmu[:])
                        sq = lnp.tile([P, D], f32, tag="sq")
                        v1 = lnp.tile([P, 1], f32, tag="v1")
                        nc.scalar.activation(sq[:], tcen[:], AF.Square,
                                             accum_out=v1[:])
                        std = lnp.tile([P, 1], f32, tag="std")
                        nc.scalar.activation(std[:], v1[:], AF.Sqrt,
                                             bias=eps_t[:], scale=1.0 / D)
                        rstd = lnp.tile([P, 1], f32, tag="rstd")
                        nc.vector.reciprocal(rstd[:], std[:])
                        out_t = lnp.tile([P, D], f32, tag="out_t")
                        nc.vector.tensor_scalar_mul(out_t[:], tcen[:], rstd[:])
                        if has_gamma:
                            nc.vector.tensor_mul(out_t[:], out_t[:], gamma_b[:])
                        if has_beta:
                            nc.vector.tensor_add(out_t[:], out_t[:], beta_b[:])
                        nc.sync.dma_start(y[qs * P:(qs + 1) * P, :], out_t[:])

        if bench_reps and _NO_AG:
            with tc.For_i(0, bench_reps, 1):
                emit_body(kv_alls[0])
        else:
            for i in range(max(1, bench_reps)):
                emit_body(kv_alls[i])
    nc.compile()
    return nc


_NC_CACHE: dict = {}


def _get_nc(flags, bench_reps: int = 0):
    key = (flags, bench_reps, _N_ACT, _NO_AG, _ABL, _CTX_FP8)
    if key not in _NC_CACHE:
        _NC_CACHE[key] = _build_nc(*flags, bench_reps=bench_reps)
    return _NC_CACHE[key]


def _prep_inputs(inputs):
    x = np.ascontiguousarray(np.asarray(inputs["x"], dtype=np.float32))
    Wq = np.asarray(inputs["Wq"], dtype=np.float32)
    Wk = np.asarray(inputs["Wk"], dtype=np.float32)
    Wv = np.asarray(inputs["Wv"], dtype=np.float32)
    Wo = np.asarray(inputs["Wo"], dtype=np.float32)
    bq = np.asarray(inputs["bq"], dtype=np.float32)
    bk = np.asarray(inputs["bk"], dtype=np.float32)
    bv = np.asarray(inputs["bv"], dtype=np.float32)
    bo = np.asarray(inputs["bo"], dtype=np.float32)
    gamma = np.asarray(inputs["gamma"], dtype=np.float32)
    beta = np.asarray(inputs["beta"], dtype=np.float32)

    has_bias = bool(np.any(bq) or np.any(bk) or np.any(bv))
    has_bo = bool(np.any(bo))
    has_gamma = bool(np.any(gamma != 1.0))
    has_beta = bool(np.any(beta))
    flags = (has_bias, has_bo, has_gamma, has_beta)

    cast = lambda a: a.astype(np.float16)
    xT = np.concatenate([x.T, np.ones((1, S), np.float32)], axis=0)
    xT = cast(xT)
    wq_e = cast(np.concatenate([Wq, bq[None, :]], axis=0) * np.float32(SCPRE))
    wk_e = cast(np.concatenate([Wk, bk[None, :]], axis=0))
    wv_e = cast(np.concatenate([Wv, bv[None, :]], axis=0))
    wo_r = _round_f32r(Wo)

    shared = {
        "wq": wq_e, "wk": wk_e, "wv": wv_e, "wo": wo_r,
        "bo": bo.reshape(1, D), "gamma": gamma.reshape(1, D),
        "beta": beta.reshape(1, D),
    }
    in_maps = []
    for i in range(NCORES):
        m = dict(shared)
        m["xq"] = np.ascontiguousarray(xT[:, i * SLICE:(i + 1) * SLICE])
        m["x_slice"] = np.ascontiguousarray(x[i * SLICE:(i + 1) * SLICE, :])
        in_maps.append(m)
    return flags, in_maps


def _run(inputs, trace=False, **kw):
    flags, in_maps = _prep_inputs(inputs)
    nc = _get_nc(flags)
    res = run_bass_kernel_spmd(nc, in_maps, core_ids=list(range(NCORES)),
                               trace=trace, **kw)
    out = np.concatenate([res.results[i]["y"] for i in range(NCORES)], axis=0)
    return out, res


def kernel(**inputs) -> np.ndarray:
    out, _ = _run(inputs, trace=False)
    return out
